# revision 37
# baseline (speedup 1.0000x reference)
"""BDH (dense_transformer) Trainium2 kernel, 8-core tensor-parallel.

Sharding: core c -> head h=c//2, parity p=c%2. Within a head, the two cores
split the T=1024 query dim into 16 blocks of 64 (block B=2j+p, j=0..7) for
causal load balance. The N=4096 latent dim is host-permuted to [evens, odds]
so rope is pure elementwise between the E half and the O half. Each core
computes its head's full x_sparse/QR (needed for keys), scores only for its
own query columns (packed causal-suffix structure), yKV/y_sparse/xy/decoder
for its query columns, then two pipelined fp16 AllReduces (T-halves) combine
the per-head partial yMLP into the replicated residual stream. Next-layer
encoder matmuls for the first T-half are emitted between the two residual
updates so the PE keeps working while the second AllReduce lands.

kernel(**inputs) takes full unsharded inputs, returns full (B,T,vocab) logits.
"""
import math
import sys

sys.path.insert(0, "/opt/trn_rl_repo")

import numpy as np
import ml_dtypes

import concourse.bass as bass
import concourse.mybir as mybir
import concourse.tile as tile
from concourse import bacc
from concourse.masks import make_identity
from concourse.tile import add_dep_helper
from concourse.bass_utils import run_bass_kernel_spmd

FP32 = mybir.dt.float32
FP16 = mybir.dt.float16
BF16 = mybir.dt.bfloat16
AF = mybir.ActivationFunctionType
ALU = mybir.AluOpType

N_CORES = 8
T = 1024
D = 256
NH = 4
N = 4096
HALF = N // 2
VOCAB = 256
EPS = 1e-5
NT = N // 128
NPAIR = HALF // 128    # 16 E/O tile pairs
TT8 = T // 128
QCOLS = 512


def build(n_layer: int):
    nc = bacc.Bacc("TRN2", target_bir_lowering=False, debug=False,
                   num_devices=N_CORES)

    enc_in = nc.dram_tensor("enc", [D, N], BF16, kind="ExternalInput").ap()
    encv_in = nc.dram_tensor("encv", [D, N], BF16, kind="ExternalInput").ap()
    dec_in = nc.dram_tensor("dec", [N, D], BF16, kind="ExternalInput").ap()
    lm_in = nc.dram_tensor("lm", [D, VOCAB], BF16, kind="ExternalInput").ap()
    ctab_in = nc.dram_tensor("ctab", [HALF, T], BF16, kind="ExternalInput").ap()
    stab_in = nc.dram_tensor("stab", [HALF, T], BF16, kind="ExternalInput").ap()
    mask_in = nc.dram_tensor("mask", [128, 64], BF16, kind="ExternalInput").ap()
    m01_in = nc.dram_tensor("m01", [128, 2], FP32, kind="ExternalInput").ap()
    poff_in = nc.dram_tensor("poff", [1, 1], mybir.dt.uint32, kind="ExternalInput").ap()
    x0_in = nc.dram_tensor("x0", [T, D], FP32, kind="ExternalInput").ap()
    out = nc.dram_tensor("out", [T, VOCAB], FP32, kind="ExternalOutput").ap()

    RG = [list(range(N_CORES))]

    with tile.TileContext(nc) as tc:
        regs = nc.alloc_registers("qoff")
        nc.regs_load(regs, poff_in[0:1, 0:1])
        qoff = nc.snap(regs, donate=True, min_val=0, max_val=64)

        import contextlib
        ctx = contextlib.ExitStack()
        with ctx:
            singles = ctx.enter_context(tc.tile_pool(name="singles", bufs=1))
            big = ctx.enter_context(tc.tile_pool(name="big", bufs=1))
            wide = ctx.enter_context(tc.tile_pool(name="wide", bufs=2))
            tmpp = ctx.enter_context(tc.tile_pool(name="tmpp", bufs=1))
            small = ctx.enter_context(tc.tile_pool(name="small", bufs=2))
            stat = ctx.enter_context(tc.tile_pool(name="stat", bufs=3))
            ps = ctx.enter_context(tc.tile_pool(name="ps", bufs=1, space="PSUM"))
            dramp = ctx.enter_context(tc.tile_pool(name="dramp", bufs=2, space="DRAM"))
            spillp = ctx.enter_context(tc.tile_pool(name="spillp", bufs=34, space="DRAM"))

            # ---- persistent weights ----
            enc_sb = singles.tile([128, 2, N], BF16)
            encv_sb = singles.tile([128, 2, N], BF16)
            dec_sb = singles.tile([128, NT, D], BF16)
            lm_sb = singles.tile([128, 2, VOCAB], BF16)
            mask_sb = singles.tile([128, 64], BF16)
            m01_sb = singles.tile([128, 2], FP32)
            eps_sb = singles.tile([128, 1], FP32)
            ident = singles.tile([128, 128], BF16)
            nc.sync.dma_start(out=enc_sb, in_=enc_in.rearrange("(kt p) n -> p kt n", p=128))
            nc.sync.dma_start(out=encv_sb, in_=encv_in.rearrange("(kt p) n -> p kt n", p=128))
            nc.sync.dma_start(out=dec_sb, in_=dec_in.rearrange("(nt p) d -> p nt d", p=128))
            nc.sync.dma_start(out=lm_sb, in_=lm_in.rearrange("(kt p) v -> p kt v", p=128))
            nc.sync.dma_start(out=mask_sb, in_=mask_in)
            nc.sync.dma_start(out=m01_sb, in_=m01_in)
            nc.vector.memset(eps_sb, EPS)
            make_identity(nc, ident)

            # ---- persistent activations ----
            xbf_sb = big.tile([128, TT8, D], BF16)
            xT_lo = big.tile([128, 2, 512], BF16)   # t cols 0..511 (d-part)
            xT_hi = big.tile([128, 2, 512], BF16)   # t cols 512..1023
            qr_sb = big.tile([128, NT, T], BF16)
            a_sb = big.tile([128, TT8, QCOLS], BF16)
            ykv_sb = big.tile([128, 4, D], BF16)
            ykvT_sb = big.tile([128, 2, QCOLS], BF16)

            def layernorm(dst, src, tag=""):
                stats = stat.tile([128, 6], FP32, tag="lnstats" + tag)
                mv = stat.tile([128, 2], FP32, tag="lnmv" + tag)
                nc.vector.bn_stats(out=stats, in_=src)
                nc.vector.bn_aggr(out=mv, in_=stats)
                std_t = stat.tile([128, 1], FP32, tag="lnstd" + tag)
                nc.scalar.activation(out=std_t, in_=mv[:, 1:2], func=AF.Sqrt,
                                     bias=eps_sb, scale=1.0)
                rstd = stat.tile([128, 1], FP32, tag="lnrstd" + tag)
                nc.vector.reciprocal(out=rstd, in_=std_t)
                negmr = stat.tile([128, 1], FP32, tag="lnnegmr" + tag)
                nc.vector.tensor_scalar(out=negmr, in0=mv[:, 0:1], scalar1=rstd,
                                        scalar2=-1.0, op0=ALU.mult, op1=ALU.mult)
                nc.scalar.activation(out=dst, in_=src, func=AF.Identity,
                                     bias=negmr, scale=rstd)

            def x_finalize(ti):
                xT = xT_lo if ti < 4 else xT_hi
                base = 128 * (ti % 4)
                for dh in range(2):
                    pst = ps.tile([128, 128], BF16, tag=f"b{6 + dh}", name=f"tp{ti}_{dh}")
                    nc.tensor.transpose(pst[:], xbf_sb[:, ti, 128 * dh:128 * dh + 128], ident[:])
                    nc.scalar.copy(out=xT[:, dh, base:base + 128], in_=pst[:])

            spill_map = {}
            GS = 4            # pairs per rope group
            NG = NPAIR // GS  # 4 groups per chunk

            def phaseA(ell, ch, unit0, cols=(0, 512), slots=None, fused=None,
                       rot=tuple(range(8)), groups=None):
                """x_sparse+rope for t-chunk ch over query-column range cols, in
                groups of GS=4 n-tile pairs so the rope runs as wide strided DVE
                ops and the spill/table DMAs batch 4 pairs per descriptor. unit0
                offsets the psum tag rotation. slots: optional {g: callback}
                invoked before group g (x_update interleaving). fused: optional
                (half, s_ps) - score matmuls for that causal T-half accumulate
                into the s_ps psum slices as each group completes."""
                xTc = xT_lo if ch == 0 else xT_hi
                c0, cw = cols
                for g in (range(NG) if groups is None else groups):
                    if slots is not None and g in slots:
                        slots[g]()
                    xsE2 = wide.tile([128, GS, cw], BF16, tag="xsE")
                    xsO2 = wide.tile([128, GS, cw], BF16, tag="xsO")
                    if True:
                        for gi in range(GS):
                            i = GS * g + gi
                            u = (unit0 + 2 * i) % len(rot)
                            tagE = f"b{rot[u]}"
                            tagO = f"b{rot[(u + 1) % len(rot)]}"
                            psE = ps.tile([128, cw], FP32, tag=tagE, name=f"psE{ell}_{ch}_{c0}_{i}")
                            psO = ps.tile([128, cw], FP32, tag=tagO, name=f"psO{ell}_{ch}_{c0}_{i}")
                            for kt in range(2):
                                nc.tensor.matmul(psE[:], enc_sb[:, kt, 128 * i:128 * i + 128],
                                                 xTc[:, kt, c0:c0 + cw], start=(kt == 0), stop=(kt == 1))
                            for kt in range(2):
                                nc.tensor.matmul(psO[:], enc_sb[:, kt, HALF + 128 * i:HALF + 128 * i + 128],
                                                 xTc[:, kt, c0:c0 + cw], start=(kt == 0), stop=(kt == 1))
                            nc.scalar.activation(out=xsE2[:, gi, :], in_=psE[:], func=AF.Relu)
                            nc.scalar.activation(out=xsO2[:, gi, :], in_=psO[:], func=AF.Relu)
                    if ch == 0 and c0 == 0:
                        spill_map[g] = spillp.tile([128, GS, QCOLS], BF16, tag="xsq",
                                                   name=f"sp{ell}_E{g}")
                        spill_map[NG + g] = spillp.tile([128, GS, QCOLS], BF16, tag="xsq",
                                                        name=f"sp{ell}_O{g}")
                    q0 = 256 * ch + c0 // 2
                    for half, xs2 in ((0, xsE2), (1, xsO2)):
                        sp = spill_map[NG * half + g]
                        nc.gpsimd.dma_start(
                            out=sp[:, :, q0:q0 + cw // 2].rearrange(
                                "p g (b w) -> p g b w", w=64),
                            in_=xs2[:].rearrange("p g (b w) -> p g b w", w=128)[:, :, :, bass.ds(qoff, 64)])
                    ct2 = small.tile([128, GS, cw], BF16, tag="ctab")
                    st2 = small.tile([128, GS, cw], BF16, tag="stab")
                    tc0 = 512 * ch + c0
                    nc.sync.dma_start(
                        out=ct2, in_=ctab_in[512 * g:512 * g + 512, tc0:tc0 + cw]
                        .rearrange("(g p) w -> p g w", p=128))
                    nc.sync.dma_start(
                        out=st2, in_=stab_in[512 * g:512 * g + 512, tc0:tc0 + cw]
                        .rearrange("(g p) w -> p g w", p=128))
                    qrE = qr_sb[:, GS * g:GS * g + GS, tc0:tc0 + cw]
                    qrO = qr_sb[:, 16 + GS * g:16 + GS * g + GS, tc0:tc0 + cw]
                    tB = tmpp.tile([128, GS, cw], BF16, tag="tt")
                    nc.vector.tensor_tensor(out=qrO, in0=xsO2[:], in1=ct2[:], op=ALU.mult)
                    nc.vector.tensor_tensor(out=tB, in0=xsE2[:], in1=st2[:], op=ALU.mult)
                    nc.vector.tensor_tensor(out=qrO, in0=qrO, in1=tB, op=ALU.add)
                    tD = tmpp.tile([128, GS, cw], BF16, tag="tt")
                    nc.vector.tensor_tensor(out=qrE, in0=xsE2[:], in1=ct2[:], op=ALU.mult)
                    nc.vector.tensor_tensor(out=tD, in0=xsO2[:], in1=st2[:], op=ALU.mult)
                    nc.vector.tensor_tensor(out=qrE, in0=qrE, in1=tD, op=ALU.subtract)
                    if fused is not None:
                        h, s_ps = fused
                        tiles = ([GS * g + k for k in range(GS)]
                                 + [16 + GS * g + k for k in range(GS)])
                        bmax = 4 * (h + 1)
                        for tile_n in tiles:
                            for t in range(4 * h + 4):
                                b0 = max(t, 4 * h)
                                rhs = qr_sb[:, tile_n, :].rearrange(
                                    "p (b w) -> p b w", w=128)[:, b0:bmax, bass.ds(qoff, 64)]
                                # s_ps packs two t-regions per psum bank; start
                                # (which clears has_written for the WHOLE bank)
                                # may only be set on the bank's first matmul.
                                # The odd-t region's first write then overwrites
                                # (its has_written bits are clear) and later
                                # matmuls accumulate.
                                nc.tensor.matmul(
                                    s_ps[t].rearrange("p (b w) -> p b w", w=64),
                                    qr_sb[:, tile_n, 128 * t:128 * t + 128],
                                    rhs, start=(g == 0 and tile_n == 0 and t % 2 == 0),
                                    stop=(g == NG - 1 and tile_n == 16 + NPAIR - 1))
                if slots is not None and NG in slots:
                    slots[NG]()

            # warmup collective: absorbs the one-time global sync barrier
            wu_in = dramp.tile([512, D], FP16, tag="wuin")
            wu_out = dramp.tile([512, D], FP16, tag="wuout")
            wu_sb = singles.tile([128, 4 * D], FP16)
            nc.vector.memset(wu_sb, 0.0)
            nc.sync.dma_start(out=wu_in[:].rearrange("(a p) d -> p a d", p=128),
                              in_=wu_sb[:].rearrange("p (a d) -> p a d", d=D))
            nc.gpsimd.collective_compute("AllReduce", ALU.add, replica_groups=RG,
                                         ins=[wu_in.opt()], outs=[wu_out.opt()])

            # ---- prologue ----
            # a_sb's strict-upper zero region [0, 64t) per t-block is never
            # rewritten by the per-layer score assembly, so zero it once.
            nc.vector.memset(a_sb[:], 0.0)
            for ti in range(TT8):
                raw = small.tile([128, D], FP32, tag="x0raw")
                nc.sync.dma_start(out=raw, in_=x0_in[128 * ti:128 * ti + 128, :])
                layernorm(xbf_sb[:, ti, :], raw)
                x_finalize(ti)

            def make_sps0(ell):
                pa = ps.tile([128, 448], FP32, tag="b4", name=f"sps0a{ell}")
                pb = ps.tile([128, 192], FP32, tag="b5", name=f"sps0b{ell}")
                return {0: pa[:, 0:256], 1: pa[:, 256:448],
                        2: pb[:, 0:128], 3: pb[:, 128:192]}

            def make_sps1(ell):
                p01 = ps.tile([128, 512], FP32, tag="b0", name=f"sps1a{ell}")
                p23 = ps.tile([128, 512], FP32, tag="b1", name=f"sps1b{ell}")
                p45 = ps.tile([128, 448], FP32, tag="b2", name=f"sps1c{ell}")
                p67 = ps.tile([128, 192], FP32, tag="b3", name=f"sps1d{ell}")
                return {0: p01[:, 0:256], 1: p01[:, 256:512],
                        2: p23[:, 0:256], 3: p23[:, 256:512],
                        4: p45[:, 0:256], 5: p45[:, 256:448],
                        6: p67[:, 0:128], 7: p67[:, 128:192]}

            def assembly(h, s_ps):
                """a_sb local-query columns [256h, 256h+256) from packed score
                psums: masked diagonal 64-block + causal-suffix copy."""
                if h == 0:
                    for t in range(4):
                        w = (4 - t) * 64
                        nc.vector.tensor_tensor(out=a_sb[:, t, 64 * t:64 * t + 64],
                                                in0=s_ps[t][:, 0:64], in1=mask_sb,
                                                op=ALU.mult)
                        if w > 64:
                            nc.scalar.copy(out=a_sb[:, t, 64 * t + 64:256],
                                           in_=s_ps[t][:, 64:w])
                else:
                    for t in range(4):
                        nc.scalar.copy(out=a_sb[:, t, 256:512], in_=s_ps[t][:, 0:256])
                    for t in range(4, 8):
                        w = (8 - t) * 64
                        nc.vector.tensor_tensor(out=a_sb[:, t, 64 * t:64 * t + 64],
                                                in0=s_ps[t][:, 0:64], in1=mask_sb,
                                                op=ALU.mult)
                        if w > 64:
                            nc.scalar.copy(out=a_sb[:, t, 64 * t + 64:512],
                                           in_=s_ps[t][:, 64:w])

            def phaseC_k(k, ell):
                psy = ps.tile([128, D], FP32, tag=f"b{4 * (k // 2) + (k % 2)}",
                              name=f"ykvps{ell}_{k}")
                tmax = 2 * k + 1
                for t in range(tmax + 1):
                    nc.tensor.matmul(psy[:], a_sb[:, t, 128 * k:128 * k + 128],
                                     xbf_sb[:, t, :], start=(t == 0), stop=(t == tmax))
                layernorm(ykv_sb[:, k, :], psy[:])
                for dh in range(2):
                    pst = ps.tile([128, 128], BF16, tag=f"b{6 + dh}",
                                  name=f"ykvT{ell}_{k}_{dh}")
                    nc.tensor.transpose(pst[:], ykv_sb[:, k, 128 * dh:128 * dh + 128],
                                        ident[:])
                    nc.scalar.copy(out=ykvT_sb[:, dh, 128 * k:128 * k + 128], in_=pst[:])

            def phaseC(h, ell):
                for k in (2 * h, 2 * h + 1):
                    phaseC_k(k, ell)

            def phaseD(h, ell, psmlp, upds=(), upd_at=(), dr=(0, 1)):
                """upds[i] is emitted before loop iteration upd_at[i]; an
                x_update whose AllReduce may still be in flight head-of-line
                blocks the in-order queues, so slots sit late enough that the
                collective has drained by the time the queues reach them."""
                q0 = 256 * h
                xsq4 = None
                for idx, jp in enumerate(range(0, NT, 2)):
                    if idx in upd_at:
                        upds[upd_at.index(idx)]()
                    ys2 = small.tile([128, 2, D], BF16, tag="ys")
                    # two y_sparse chains share one psum bank (has_written:
                    # start only on the bank's first matmul)
                    psy = ps.tile([128, 2, D], FP32, tag=f"b{dr[(jp // 2) % len(dr)]}",
                                  name=f"ysps{ell}_{h}_{jp}")
                    for jj in range(2):
                        j = jp + jj
                        for kt in range(2):
                            nc.tensor.matmul(psy[:, jj, :], encv_sb[:, kt, 128 * j:128 * j + 128],
                                             ykvT_sb[:, kt, q0:q0 + 256],
                                             start=(kt == 0 and jj == 0), stop=(kt == 1))
                    nc.scalar.activation(out=ys2[:], in_=psy[:], func=AF.Relu)
                    if jp < 16:
                        gsp, sl = jp // GS, jp % GS
                    else:
                        gsp, sl = NG + (jp - 16) // GS, (jp - 16) % GS
                    if sl % 4 == 0:
                        xsq4 = small.tile([128, GS, D], BF16, tag="xsqb")
                        nc.sync.dma_start(out=xsq4,
                                          in_=spill_map[gsp][:, :, q0:q0 + 256])
                    xy2 = small.tile([128, 2, D], BF16, tag="xy")
                    nc.vector.tensor_tensor(out=xy2, in0=ys2,
                                            in1=xsq4[:, sl % 4:sl % 4 + 2, :], op=ALU.mult)
                    for jj in range(2):
                        for kk in range(2):
                            nc.tensor.matmul(psmlp[:, kk, :],
                                             xy2[:, jj, 128 * kk:128 * kk + 128],
                                             dec_sb[:, jp + jj, :],
                                             start=(jp + jj == 0 and kk == 0),
                                             stop=(jp + jj == NT - 1))


            def phaseE(h, ell, psmlp):
                """fp16 AllReduce of this half's yMLP partial (global rows
                [512h, 512h+512)); foreign-parity 64-blocks written as zeros."""
                arin = dramp.tile([512, D], FP16, tag=f"ar{h}in", name=f"arin{ell}_{h}")
                arout = dramp.tile([512, D], FP16, tag=f"ar{h}out", name=f"arout{ell}_{h}")
                wds = []
                for P in (0, 1):
                    for kk in range(2):
                        ym = stat.tile([128, D], FP16, tag="ymsk")
                        nc.vector.tensor_scalar_mul(out=ym, in0=psmlp[:, kk, :],
                                                    scalar1=m01_sb[:, P:P + 1])
                        for aa in range(2):
                            out_ap = bass.AP(
                                tensor=arin.tensor,
                                offset=arin.offset + (256 * kk + 64 * P + 128 * aa) * D,
                                ap=[[D, 64], [1, D]],
                            )
                            eng = (nc.sync, nc.gpsimd)[aa]
                            wds.append(eng.dma_start(
                                out=out_ap, in_=ym[64 * aa:64 * aa + 64, :]))
                cc = nc.gpsimd.collective_compute(
                    "AllReduce", ALU.add, replica_groups=RG,
                    ins=[arin.opt()], outs=[arout.opt()])
                for w in wds:
                    add_dep_helper(cc.ins, w.ins, sync=True, reason="w->ar")
                return cc, arout

            def x_update(ti, ccpair):
                cc, arout = ccpair
                rd = stat.tile([128, D], FP16, tag="ymrd")
                # issue on the Vector queue: the AllReduce-completion wait then
                # head-of-line blocks only work that depends on this update,
                # never the table/spill DMA stream on Sync.
                rdma = nc.sync.dma_start(
                    out=rd, in_=arout[128 * (ti % 4):128 * (ti % 4) + 128, :])
                add_dep_helper(rdma.ins, cc.ins, sync=True, reason="ar->r")
                lnym = stat.tile([128, D], FP32, tag="lnym")
                layernorm(lnym, rd)
                xn = stat.tile([128, D], FP32, tag="xn")
                nc.vector.tensor_add(out=xn, in0=xbf_sb[:, ti, :], in1=lnym)
                layernorm(xbf_sb[:, ti, :], xn)
                x_finalize(ti)

            # ---- layers, software-pipelined over causal T-halves ----
            # Half 0 (queries t<512) only attends keys t<512, so its entire
            # pipeline A..E runs and its AllReduce launches ~half a layer before
            # the layer ends; both collectives are consumed a full half-layer
            # after launch, hiding the ~15us collective latency completely.
            ar1_prev = None
            for ell in range(n_layer):
                # -- half 0 --
                s_ps0 = make_sps0(ell)
                phaseA(ell, 0, 0, cols=(0, 512), fused=(0, s_ps0),
                       rot=(0, 1, 2, 3, 6, 7))
                assembly(0, s_ps0)
                if ell > 0:
                    for ti in (4, 5, 6, 7):
                        x_update(ti, ar1_prev)
                phaseC(0, ell)
                # A1's encoder+rope only needs the row-4..7 updates, so its
                # groups interleave into the PE-light D0 window; its score
                # matmuls run as a standalone pass afterwards.
                a1g = tuple(
                    (lambda g=g: phaseA(ell, 1, 0, cols=(0, 512),
                                        rot=(2, 3, 5, 6, 7), groups=(g,)))
                    for g in range(NG))
                psmlp0 = ps.tile([128, 2, D], FP32, tag="b4", name=f"psmlp{ell}_0")
                phaseD(0, ell, psmlp0, a1g[:3], upd_at=(4, 8, 12))
                cc0 = phaseE(0, ell, psmlp0)
                # -- half 1 --
                a1g[3]()
                s_ps1 = make_sps1(ell)
                for tile_n in range(NT):
                    for t in range(8):
                        b0 = max(t, 4)
                        rhs = qr_sb[:, tile_n, :].rearrange(
                            "p (b w) -> p b w", w=128)[:, b0:TT8, bass.ds(qoff, 64)]
                        nc.tensor.matmul(
                            s_ps1[t].rearrange("p (b w) -> p b w", w=64),
                            qr_sb[:, tile_n, 128 * t:128 * t + 128],
                            rhs, start=(tile_n == 0 and t % 2 == 0),
                            stop=(tile_n == NT - 1))
                assembly(1, s_ps1)
                phaseC(1, ell)
                psmlp1 = ps.tile([128, 2, D], FP32, tag="b4", name=f"psmlp{ell}_1")
                phaseD(1, ell, psmlp1,
                       tuple((lambda ti=ti, cp=cc0: x_update(ti, cp)) for ti in (0, 1, 2, 3)),
                       upd_at=(0, 4, 8, 12))
                ar1_prev = phaseE(1, ell, psmlp1)

            # ---- logits ----
            def logits(ti):
                psl = ps.tile([128, VOCAB], FP32, tag=f"b{ti % 2}", name=f"lgps{ti}")
                xT = xT_lo if ti < 4 else xT_hi
                base = 128 * (ti % 4)
                for kt in range(2):
                    nc.tensor.matmul(psl[:], xT[:, kt, base:base + 128],
                                     lm_sb[:, kt, :], start=(kt == 0), stop=(kt == 1))
                lg = small.tile([128, VOCAB], FP32, tag="lg")
                nc.scalar.copy(out=lg, in_=psl[:])
                nc.sync.dma_start(out=out[128 * ti:128 * ti + 128, :], in_=lg)

            for ti in range(4):
                logits(ti)
            for ti in range(4, 8):
                x_update(ti, ar1_prev)
            for ti in range(4, 8):
                logits(ti)

    nc.compile()
    return nc


def _host_prep(idx, embed_w, encoder, encoder_v, decoder, lm_head):
    idx = np.asarray(idx)
    B, Tt = idx.shape
    assert B == 1 and Tt == T
    perm = np.concatenate([np.arange(0, N, 2), np.arange(1, N, 2)])

    def bf(x):
        return np.ascontiguousarray(x).astype(ml_dtypes.bfloat16)

    theta = 2.0 ** 16
    q = np.floor(np.arange(N, dtype=np.float32) / 2.0) * 2.0
    freqs = (1.0 / (theta ** (q / np.float32(N))) / np.float32(2.0 * math.pi)).astype(np.float32)
    phases = np.arange(T, dtype=np.float32)[:, None] * freqs[None, 0::2]
    ph = np.float32(2.0 * math.pi) * (phases % np.float32(1.0))
    ctab = bf(np.cos(ph).T)
    stab = bf(np.sin(ph).T)

    x0 = np.ascontiguousarray(embed_w[idx[0]]).astype(np.float32)
    lm_bf = bf(lm_head)

    r = np.arange(128)[:, None]
    c64 = np.arange(64)[None, :]
    in_maps = []
    for c in range(N_CORES):
        h, p = c // 2, c % 2
        m01 = np.zeros((128, 2), np.float32)
        m01[:, 0] = 1.0 - p
        m01[:, 1] = p
        in_maps.append({
            "enc": bf(encoder[h][:, perm]),
            "encv": bf(encoder_v[h][:, perm]),
            "dec": bf(decoder.reshape(NH, N, D)[h][perm, :]),
            "lm": lm_bf,
            "ctab": ctab,
            "stab": stab,
            "mask": bf((r < c64 + 64 * p).astype(np.float32)),
            "m01": m01,
            "poff": np.array([[64 * p]], dtype=np.uint32),
            "x0": x0,
        })
    return in_maps


_NC_CACHE = {}


def kernel(idx, n_layer, embed_w, encoder, encoder_v, decoder, lm_head,
           _trace=False, _trace_kwargs=None):
    n_layer = int(np.asarray(n_layer))
    idx = np.asarray(idx)
    B = idx.shape[0]
    if n_layer not in _NC_CACHE:
        _NC_CACHE[n_layer] = build(n_layer)
    nc = _NC_CACHE[n_layer]
    in_maps = _host_prep(idx, np.asarray(embed_w, np.float32),
                         np.asarray(encoder, np.float32),
                         np.asarray(encoder_v, np.float32),
                         np.asarray(decoder, np.float32),
                         np.asarray(lm_head, np.float32))
    kw = {}
    if _trace:
        kw = dict(trace=True, **(_trace_kwargs or {}))
    res = run_bass_kernel_spmd(nc, in_maps, core_ids=list(range(N_CORES)), **kw)
    logits = res.results[0]["out"].astype(np.float32).reshape(B, T, VOCAB)
    kernel._last_results = res
    return logits



# revision 38
# speedup vs baseline: 1.1526x; 1.1526x over previous
"""BDH (dense_transformer) Trainium2 kernel, 8-core tensor-parallel.

Sharding: core c -> head h=c//2, parity p=c%2. Within a head, the two cores
split the T=1024 query dim into 16 blocks of 64 (block B=2j+p, j=0..7) for
causal load balance. The N=4096 latent dim is host-permuted to [evens, odds]
so rope is pure elementwise between the E half and the O half.

Each layer is software-pipelined over causal T-halves: queries t<512 attend
only keys t<512, so half 0's full pipeline (encoder+rope -> fused scores ->
yKV -> y_sparse/decoder) finishes and launches its fp16 AllReduce roughly
half a layer before the layer ends; both collectives are consumed a full
half-layer after launch, hiding the ~15-30us mesh-collective latency behind
compute. Half 1's encoder+rope (which only needs the row-4..7 residual
updates) is interleaved into half 0's PE-light decoder window; its score
matmuls run as a standalone pass. Residual updates for rows 0..3 / 4..7 are
slotted where their AllReduce is provably complete, because a dependent DMA
wait head-of-line blocks the whole in-order engine queue hosting it. Score
accumulators pack two causal-suffix regions per PSUM bank ('start' may only
be set on a bank's first matmul - it clears has_written bank-wide). The
residual stream lives in bf16; x_sparse spills to DRAM in batched 4-tile
descriptors and is re-read the same way.

kernel(**inputs) takes full unsharded inputs, returns full (B,T,vocab) logits.
"""
import math
import sys

sys.path.insert(0, "/opt/trn_rl_repo")

import numpy as np
import ml_dtypes

import concourse.bass as bass
import concourse.mybir as mybir
import concourse.tile as tile
from concourse import bacc
from concourse.masks import make_identity
from concourse.tile import add_dep_helper
from concourse.bass_utils import run_bass_kernel_spmd

FP32 = mybir.dt.float32
FP16 = mybir.dt.float16
BF16 = mybir.dt.bfloat16
AF = mybir.ActivationFunctionType
ALU = mybir.AluOpType

N_CORES = 8
T = 1024
D = 256
NH = 4
N = 4096
HALF = N // 2
VOCAB = 256
EPS = 1e-5
NT = N // 128
NPAIR = HALF // 128    # 16 E/O tile pairs
TT8 = T // 128
QCOLS = 512


def build(n_layer: int):
    nc = bacc.Bacc("TRN2", target_bir_lowering=False, debug=False,
                   num_devices=N_CORES)

    enc_in = nc.dram_tensor("enc", [D, N], BF16, kind="ExternalInput").ap()
    encv_in = nc.dram_tensor("encv", [D, N], BF16, kind="ExternalInput").ap()
    dec_in = nc.dram_tensor("dec", [N, D], BF16, kind="ExternalInput").ap()
    lm_in = nc.dram_tensor("lm", [D, VOCAB], BF16, kind="ExternalInput").ap()
    ctab_in = nc.dram_tensor("ctab", [HALF, T], BF16, kind="ExternalInput").ap()
    stab_in = nc.dram_tensor("stab", [HALF, T], BF16, kind="ExternalInput").ap()
    mask_in = nc.dram_tensor("mask", [128, 64], BF16, kind="ExternalInput").ap()
    m01_in = nc.dram_tensor("m01", [128, 2], FP32, kind="ExternalInput").ap()
    poff_in = nc.dram_tensor("poff", [1, 1], mybir.dt.uint32, kind="ExternalInput").ap()
    x0_in = nc.dram_tensor("x0", [T, D], FP32, kind="ExternalInput").ap()
    out = nc.dram_tensor("out", [T, VOCAB], FP32, kind="ExternalOutput").ap()

    RG = [list(range(N_CORES))]

    with tile.TileContext(nc) as tc:
        regs = nc.alloc_registers("qoff")
        nc.regs_load(regs, poff_in[0:1, 0:1])
        qoff = nc.snap(regs, donate=True, min_val=0, max_val=64)

        import contextlib
        ctx = contextlib.ExitStack()
        with ctx:
            singles = ctx.enter_context(tc.tile_pool(name="singles", bufs=1))
            big = ctx.enter_context(tc.tile_pool(name="big", bufs=1))
            wide = ctx.enter_context(tc.tile_pool(name="wide", bufs=2))
            tmpp = ctx.enter_context(tc.tile_pool(name="tmpp", bufs=1))
            small = ctx.enter_context(tc.tile_pool(name="small", bufs=2))
            stat = ctx.enter_context(tc.tile_pool(name="stat", bufs=3))
            ps = ctx.enter_context(tc.tile_pool(name="ps", bufs=1, space="PSUM"))
            dramp = ctx.enter_context(tc.tile_pool(name="dramp", bufs=2, space="DRAM"))
            spillp = ctx.enter_context(tc.tile_pool(name="spillp", bufs=34, space="DRAM"))

            # ---- persistent weights ----
            enc_sb = singles.tile([128, 2, N], BF16)
            encv_sb = singles.tile([128, 2, N], BF16)
            dec_sb = singles.tile([128, NT, D], BF16)
            lm_sb = singles.tile([128, 2, VOCAB], BF16)
            mask_sb = singles.tile([128, 64], BF16)
            m01_sb = singles.tile([128, 2], FP32)
            eps_sb = singles.tile([128, 1], FP32)
            ident = singles.tile([128, 128], BF16)
            nc.sync.dma_start(out=enc_sb, in_=enc_in.rearrange("(kt p) n -> p kt n", p=128))
            nc.sync.dma_start(out=encv_sb, in_=encv_in.rearrange("(kt p) n -> p kt n", p=128))
            nc.sync.dma_start(out=dec_sb, in_=dec_in.rearrange("(nt p) d -> p nt d", p=128))
            nc.sync.dma_start(out=lm_sb, in_=lm_in.rearrange("(kt p) v -> p kt v", p=128))
            nc.sync.dma_start(out=mask_sb, in_=mask_in)
            nc.sync.dma_start(out=m01_sb, in_=m01_in)
            nc.vector.memset(eps_sb, EPS)
            make_identity(nc, ident)

            # ---- persistent activations ----
            xbf_sb = big.tile([128, TT8, D], BF16)
            xT_lo = big.tile([128, 2, 512], BF16)   # t cols 0..511 (d-part)
            xT_hi = big.tile([128, 2, 512], BF16)   # t cols 512..1023
            qr_sb = big.tile([128, NT, T], BF16)
            a_sb = big.tile([128, TT8, QCOLS], BF16)
            ykv_sb = big.tile([128, 4, D], BF16)
            ykvT_sb = big.tile([128, 2, QCOLS], BF16)

            def layernorm(dst, src, tag=""):
                stats = stat.tile([128, 6], FP32, tag="lnstats" + tag)
                mv = stat.tile([128, 2], FP32, tag="lnmv" + tag)
                nc.vector.bn_stats(out=stats, in_=src)
                nc.vector.bn_aggr(out=mv, in_=stats)
                std_t = stat.tile([128, 1], FP32, tag="lnstd" + tag)
                nc.scalar.activation(out=std_t, in_=mv[:, 1:2], func=AF.Sqrt,
                                     bias=eps_sb, scale=1.0)
                rstd = stat.tile([128, 1], FP32, tag="lnrstd" + tag)
                nc.vector.reciprocal(out=rstd, in_=std_t)
                negmr = stat.tile([128, 1], FP32, tag="lnnegmr" + tag)
                nc.vector.tensor_scalar(out=negmr, in0=mv[:, 0:1], scalar1=rstd,
                                        scalar2=-1.0, op0=ALU.mult, op1=ALU.mult)
                nc.scalar.activation(out=dst, in_=src, func=AF.Identity,
                                     bias=negmr, scale=rstd)

            def x_finalize(ti):
                xT = xT_lo if ti < 4 else xT_hi
                base = 128 * (ti % 4)
                for dh in range(2):
                    pst = ps.tile([128, 128], BF16, tag=f"b{6 + dh}", name=f"tp{ti}_{dh}")
                    nc.tensor.transpose(pst[:], xbf_sb[:, ti, 128 * dh:128 * dh + 128], ident[:])
                    nc.scalar.copy(out=xT[:, dh, base:base + 128], in_=pst[:])

            spill_map = {}
            GS = 4            # pairs per rope group
            NG = NPAIR // GS  # 4 groups per chunk

            def phaseA(ell, ch, unit0, cols=(0, 512), slots=None, fused=None,
                       rot=tuple(range(8)), groups=None):
                """x_sparse+rope for t-chunk ch over query-column range cols, in
                groups of GS=4 n-tile pairs so the rope runs as wide strided DVE
                ops and the spill/table DMAs batch 4 pairs per descriptor. unit0
                offsets the psum tag rotation. slots: optional {g: callback}
                invoked before group g (x_update interleaving). fused: optional
                (half, s_ps) - score matmuls for that causal T-half accumulate
                into the s_ps psum slices as each group completes."""
                xTc = xT_lo if ch == 0 else xT_hi
                c0, cw = cols
                for g in (range(NG) if groups is None else groups):
                    if slots is not None and g in slots:
                        slots[g]()
                    xsE2 = wide.tile([128, GS, cw], BF16, tag="xsE")
                    xsO2 = wide.tile([128, GS, cw], BF16, tag="xsO")
                    if True:
                        for gi in range(GS):
                            i = GS * g + gi
                            u = (unit0 + 2 * i) % len(rot)
                            tagE = f"b{rot[u]}"
                            tagO = f"b{rot[(u + 1) % len(rot)]}"
                            psE = ps.tile([128, cw], FP32, tag=tagE, name=f"psE{ell}_{ch}_{c0}_{i}")
                            psO = ps.tile([128, cw], FP32, tag=tagO, name=f"psO{ell}_{ch}_{c0}_{i}")
                            for kt in range(2):
                                nc.tensor.matmul(psE[:], enc_sb[:, kt, 128 * i:128 * i + 128],
                                                 xTc[:, kt, c0:c0 + cw], start=(kt == 0), stop=(kt == 1))
                            for kt in range(2):
                                nc.tensor.matmul(psO[:], enc_sb[:, kt, HALF + 128 * i:HALF + 128 * i + 128],
                                                 xTc[:, kt, c0:c0 + cw], start=(kt == 0), stop=(kt == 1))
                            nc.scalar.activation(out=xsE2[:, gi, :], in_=psE[:], func=AF.Relu)
                            nc.scalar.activation(out=xsO2[:, gi, :], in_=psO[:], func=AF.Relu)
                    if ch == 0 and c0 == 0:
                        spill_map[g] = spillp.tile([128, GS, QCOLS], BF16, tag="xsq",
                                                   name=f"sp{ell}_E{g}")
                        spill_map[NG + g] = spillp.tile([128, GS, QCOLS], BF16, tag="xsq",
                                                        name=f"sp{ell}_O{g}")
                    q0 = 256 * ch + c0 // 2
                    for half, xs2 in ((0, xsE2), (1, xsO2)):
                        sp = spill_map[NG * half + g]
                        nc.gpsimd.dma_start(
                            out=sp[:, :, q0:q0 + cw // 2].rearrange(
                                "p g (b w) -> p g b w", w=64),
                            in_=xs2[:].rearrange("p g (b w) -> p g b w", w=128)[:, :, :, bass.ds(qoff, 64)])
                    ct2 = small.tile([128, GS, cw], BF16, tag="ctab")
                    st2 = small.tile([128, GS, cw], BF16, tag="stab")
                    tc0 = 512 * ch + c0
                    nc.sync.dma_start(
                        out=ct2, in_=ctab_in[512 * g:512 * g + 512, tc0:tc0 + cw]
                        .rearrange("(g p) w -> p g w", p=128))
                    nc.sync.dma_start(
                        out=st2, in_=stab_in[512 * g:512 * g + 512, tc0:tc0 + cw]
                        .rearrange("(g p) w -> p g w", p=128))
                    qrE = qr_sb[:, GS * g:GS * g + GS, tc0:tc0 + cw]
                    qrO = qr_sb[:, 16 + GS * g:16 + GS * g + GS, tc0:tc0 + cw]
                    tB = tmpp.tile([128, GS, cw], BF16, tag="tt")
                    nc.vector.tensor_tensor(out=qrO, in0=xsO2[:], in1=ct2[:], op=ALU.mult)
                    nc.vector.tensor_tensor(out=tB, in0=xsE2[:], in1=st2[:], op=ALU.mult)
                    nc.vector.tensor_tensor(out=qrO, in0=qrO, in1=tB, op=ALU.add)
                    tD = tmpp.tile([128, GS, cw], BF16, tag="tt")
                    nc.vector.tensor_tensor(out=qrE, in0=xsE2[:], in1=ct2[:], op=ALU.mult)
                    nc.vector.tensor_tensor(out=tD, in0=xsO2[:], in1=st2[:], op=ALU.mult)
                    nc.vector.tensor_tensor(out=qrE, in0=qrE, in1=tD, op=ALU.subtract)
                    if fused is not None:
                        h, s_ps = fused
                        tiles = ([GS * g + k for k in range(GS)]
                                 + [16 + GS * g + k for k in range(GS)])
                        bmax = 4 * (h + 1)
                        for tile_n in tiles:
                            for t in range(4 * h + 4):
                                b0 = max(t, 4 * h)
                                rhs = qr_sb[:, tile_n, :].rearrange(
                                    "p (b w) -> p b w", w=128)[:, b0:bmax, bass.ds(qoff, 64)]
                                # s_ps packs two t-regions per psum bank; start
                                # (which clears has_written for the WHOLE bank)
                                # may only be set on the bank's first matmul.
                                # The odd-t region's first write then overwrites
                                # (its has_written bits are clear) and later
                                # matmuls accumulate.
                                nc.tensor.matmul(
                                    s_ps[t].rearrange("p (b w) -> p b w", w=64),
                                    qr_sb[:, tile_n, 128 * t:128 * t + 128],
                                    rhs, start=(g == 0 and tile_n == 0 and t % 2 == 0),
                                    stop=(g == NG - 1 and tile_n == 16 + NPAIR - 1))
                if slots is not None and NG in slots:
                    slots[NG]()

            # warmup collective: absorbs the one-time global sync barrier
            wu_in = dramp.tile([512, D], FP16, tag="wuin")
            wu_out = dramp.tile([512, D], FP16, tag="wuout")
            wu_sb = singles.tile([128, 4 * D], FP16)
            nc.vector.memset(wu_sb, 0.0)
            nc.sync.dma_start(out=wu_in[:].rearrange("(a p) d -> p a d", p=128),
                              in_=wu_sb[:].rearrange("p (a d) -> p a d", d=D))
            nc.gpsimd.collective_compute("AllReduce", ALU.add, replica_groups=RG,
                                         ins=[wu_in.opt()], outs=[wu_out.opt()])

            # ---- prologue ----
            # a_sb's strict-upper zero region [0, 64t) per t-block is never
            # rewritten by the per-layer score assembly, so zero it once.
            nc.vector.memset(a_sb[:], 0.0)
            for ti in range(TT8):
                raw = small.tile([128, D], FP32, tag="x0raw")
                nc.sync.dma_start(out=raw, in_=x0_in[128 * ti:128 * ti + 128, :])
                layernorm(xbf_sb[:, ti, :], raw)
                x_finalize(ti)

            def make_sps0(ell):
                pa = ps.tile([128, 448], FP32, tag="b4", name=f"sps0a{ell}")
                pb = ps.tile([128, 192], FP32, tag="b5", name=f"sps0b{ell}")
                return {0: pa[:, 0:256], 1: pa[:, 256:448],
                        2: pb[:, 0:128], 3: pb[:, 128:192]}

            def make_sps1(ell):
                p01 = ps.tile([128, 512], FP32, tag="b0", name=f"sps1a{ell}")
                p23 = ps.tile([128, 512], FP32, tag="b1", name=f"sps1b{ell}")
                p45 = ps.tile([128, 448], FP32, tag="b2", name=f"sps1c{ell}")
                p67 = ps.tile([128, 192], FP32, tag="b3", name=f"sps1d{ell}")
                return {0: p01[:, 0:256], 1: p01[:, 256:512],
                        2: p23[:, 0:256], 3: p23[:, 256:512],
                        4: p45[:, 0:256], 5: p45[:, 256:448],
                        6: p67[:, 0:128], 7: p67[:, 128:192]}

            def assembly(h, s_ps):
                """a_sb local-query columns [256h, 256h+256) from packed score
                psums: masked diagonal 64-block + causal-suffix copy."""
                if h == 0:
                    for t in range(4):
                        w = (4 - t) * 64
                        nc.vector.tensor_tensor(out=a_sb[:, t, 64 * t:64 * t + 64],
                                                in0=s_ps[t][:, 0:64], in1=mask_sb,
                                                op=ALU.mult)
                        if w > 64:
                            nc.scalar.copy(out=a_sb[:, t, 64 * t + 64:256],
                                           in_=s_ps[t][:, 64:w])
                else:
                    for t in range(4):
                        nc.scalar.copy(out=a_sb[:, t, 256:512], in_=s_ps[t][:, 0:256])
                    for t in range(4, 8):
                        w = (8 - t) * 64
                        nc.vector.tensor_tensor(out=a_sb[:, t, 64 * t:64 * t + 64],
                                                in0=s_ps[t][:, 0:64], in1=mask_sb,
                                                op=ALU.mult)
                        if w > 64:
                            nc.scalar.copy(out=a_sb[:, t, 64 * t + 64:512],
                                           in_=s_ps[t][:, 64:w])

            def phaseC_k(k, ell):
                psy = ps.tile([128, D], FP32, tag=f"b{4 * (k // 2) + (k % 2)}",
                              name=f"ykvps{ell}_{k}")
                tmax = 2 * k + 1
                for t in range(tmax + 1):
                    nc.tensor.matmul(psy[:], a_sb[:, t, 128 * k:128 * k + 128],
                                     xbf_sb[:, t, :], start=(t == 0), stop=(t == tmax))
                layernorm(ykv_sb[:, k, :], psy[:])
                for dh in range(2):
                    pst = ps.tile([128, 128], BF16, tag=f"b{6 + dh}",
                                  name=f"ykvT{ell}_{k}_{dh}")
                    nc.tensor.transpose(pst[:], ykv_sb[:, k, 128 * dh:128 * dh + 128],
                                        ident[:])
                    nc.scalar.copy(out=ykvT_sb[:, dh, 128 * k:128 * k + 128], in_=pst[:])

            def phaseC(h, ell):
                for k in (2 * h, 2 * h + 1):
                    phaseC_k(k, ell)

            def phaseD(h, ell, psmlp, upds=(), upd_at=(), dr=(0, 1)):
                """upds[i] is emitted before loop iteration upd_at[i]; an
                x_update whose AllReduce may still be in flight head-of-line
                blocks the in-order queues, so slots sit late enough that the
                collective has drained by the time the queues reach them."""
                q0 = 256 * h
                xsq4 = None
                for idx, jp in enumerate(range(0, NT, 2)):
                    if idx in upd_at:
                        upds[upd_at.index(idx)]()
                    ys2 = small.tile([128, 2, D], BF16, tag="ys")
                    # two y_sparse chains share one psum bank (has_written:
                    # start only on the bank's first matmul)
                    psy = ps.tile([128, 2, D], FP32, tag=f"b{dr[(jp // 2) % len(dr)]}",
                                  name=f"ysps{ell}_{h}_{jp}")
                    for jj in range(2):
                        j = jp + jj
                        for kt in range(2):
                            nc.tensor.matmul(psy[:, jj, :], encv_sb[:, kt, 128 * j:128 * j + 128],
                                             ykvT_sb[:, kt, q0:q0 + 256],
                                             start=(kt == 0 and jj == 0), stop=(kt == 1))
                    nc.scalar.activation(out=ys2[:], in_=psy[:], func=AF.Relu)
                    if jp < 16:
                        gsp, sl = jp // GS, jp % GS
                    else:
                        gsp, sl = NG + (jp - 16) // GS, (jp - 16) % GS
                    if sl % 4 == 0:
                        xsq4 = small.tile([128, GS, D], BF16, tag="xsqb")
                        nc.sync.dma_start(out=xsq4,
                                          in_=spill_map[gsp][:, :, q0:q0 + 256])
                    xy2 = small.tile([128, 2, D], BF16, tag="xy")
                    nc.vector.tensor_tensor(out=xy2, in0=ys2,
                                            in1=xsq4[:, sl % 4:sl % 4 + 2, :], op=ALU.mult)
                    for jj in range(2):
                        for kk in range(2):
                            nc.tensor.matmul(psmlp[:, kk, :],
                                             xy2[:, jj, 128 * kk:128 * kk + 128],
                                             dec_sb[:, jp + jj, :],
                                             start=(jp + jj == 0 and kk == 0),
                                             stop=(jp + jj == NT - 1))


            def phaseE(h, ell, psmlp):
                """fp16 AllReduce of this half's yMLP partial (global rows
                [512h, 512h+512)); foreign-parity 64-blocks written as zeros."""
                arin = dramp.tile([512, D], FP16, tag=f"ar{h}in", name=f"arin{ell}_{h}")
                arout = dramp.tile([512, D], FP16, tag=f"ar{h}out", name=f"arout{ell}_{h}")
                wds = []
                for P in (0, 1):
                    for kk in range(2):
                        ym = stat.tile([128, D], FP16, tag="ymsk")
                        nc.vector.tensor_scalar_mul(out=ym, in0=psmlp[:, kk, :],
                                                    scalar1=m01_sb[:, P:P + 1])
                        for aa in range(2):
                            out_ap = bass.AP(
                                tensor=arin.tensor,
                                offset=arin.offset + (256 * kk + 64 * P + 128 * aa) * D,
                                ap=[[D, 64], [1, D]],
                            )
                            eng = (nc.sync, nc.gpsimd)[aa]
                            wds.append(eng.dma_start(
                                out=out_ap, in_=ym[64 * aa:64 * aa + 64, :]))
                cc = nc.gpsimd.collective_compute(
                    "AllReduce", ALU.add, replica_groups=RG,
                    ins=[arin.opt()], outs=[arout.opt()])
                for w in wds:
                    add_dep_helper(cc.ins, w.ins, sync=True, reason="w->ar")
                return cc, arout

            def x_update(ti, ccpair):
                cc, arout = ccpair
                rd = stat.tile([128, D], FP16, tag="ymrd")
                # issue on the Vector queue: the AllReduce-completion wait then
                # head-of-line blocks only work that depends on this update,
                # never the table/spill DMA stream on Sync.
                rdma = nc.sync.dma_start(
                    out=rd, in_=arout[128 * (ti % 4):128 * (ti % 4) + 128, :])
                add_dep_helper(rdma.ins, cc.ins, sync=True, reason="ar->r")
                lnym = stat.tile([128, D], FP32, tag="lnym")
                layernorm(lnym, rd)
                xn = stat.tile([128, D], FP32, tag="xn")
                nc.vector.tensor_add(out=xn, in0=xbf_sb[:, ti, :], in1=lnym)
                layernorm(xbf_sb[:, ti, :], xn)
                x_finalize(ti)

            # ---- layers, software-pipelined over causal T-halves ----
            # Half 0 (queries t<512) only attends keys t<512, so its entire
            # pipeline A..E runs and its AllReduce launches ~half a layer before
            # the layer ends; both collectives are consumed a full half-layer
            # after launch, hiding the ~15us collective latency completely.
            ar1_prev = None
            for ell in range(n_layer):
                # -- half 0 --
                s_ps0 = make_sps0(ell)
                phaseA(ell, 0, 0, cols=(0, 512), fused=(0, s_ps0),
                       rot=(0, 1, 2, 3, 6, 7))
                assembly(0, s_ps0)
                if ell > 0:
                    for ti in (4, 5, 6, 7):
                        x_update(ti, ar1_prev)
                phaseC(0, ell)
                # A1's encoder+rope only needs the row-4..7 updates, so its
                # groups interleave into the PE-light D0 window; its score
                # matmuls run as a standalone pass afterwards.
                a1g = tuple(
                    (lambda g=g: phaseA(ell, 1, 0, cols=(0, 512),
                                        rot=(2, 3, 5, 6, 7), groups=(g,)))
                    for g in range(NG))
                psmlp0 = ps.tile([128, 2, D], FP32, tag="b4", name=f"psmlp{ell}_0")
                phaseD(0, ell, psmlp0, a1g[:3], upd_at=(4, 8, 12))
                cc0 = phaseE(0, ell, psmlp0)
                # -- half 1 --
                a1g[3]()
                s_ps1 = make_sps1(ell)
                for tile_n in range(NT):
                    for t in range(8):
                        b0 = max(t, 4)
                        rhs = qr_sb[:, tile_n, :].rearrange(
                            "p (b w) -> p b w", w=128)[:, b0:TT8, bass.ds(qoff, 64)]
                        nc.tensor.matmul(
                            s_ps1[t].rearrange("p (b w) -> p b w", w=64),
                            qr_sb[:, tile_n, 128 * t:128 * t + 128],
                            rhs, start=(tile_n == 0 and t % 2 == 0),
                            stop=(tile_n == NT - 1))
                assembly(1, s_ps1)
                phaseC(1, ell)
                psmlp1 = ps.tile([128, 2, D], FP32, tag="b4", name=f"psmlp{ell}_1")
                phaseD(1, ell, psmlp1,
                       tuple((lambda ti=ti, cp=cc0: x_update(ti, cp)) for ti in (0, 1, 2, 3)),
                       upd_at=(0, 4, 8, 12))
                ar1_prev = phaseE(1, ell, psmlp1)

            # ---- logits ----
            def logits(ti):
                psl = ps.tile([128, VOCAB], FP32, tag=f"b{ti % 2}", name=f"lgps{ti}")
                xT = xT_lo if ti < 4 else xT_hi
                base = 128 * (ti % 4)
                for kt in range(2):
                    nc.tensor.matmul(psl[:], xT[:, kt, base:base + 128],
                                     lm_sb[:, kt, :], start=(kt == 0), stop=(kt == 1))
                lg = small.tile([128, VOCAB], FP32, tag="lg")
                nc.scalar.copy(out=lg, in_=psl[:])
                nc.sync.dma_start(out=out[128 * ti:128 * ti + 128, :], in_=lg)

            for ti in range(4):
                logits(ti)
            for ti in range(4, 8):
                x_update(ti, ar1_prev)
            for ti in range(4, 8):
                logits(ti)

    nc.compile()
    return nc


def _host_prep(idx, embed_w, encoder, encoder_v, decoder, lm_head):
    idx = np.asarray(idx)
    B, Tt = idx.shape
    assert B == 1 and Tt == T
    perm = np.concatenate([np.arange(0, N, 2), np.arange(1, N, 2)])

    def bf(x):
        return np.ascontiguousarray(x).astype(ml_dtypes.bfloat16)

    theta = 2.0 ** 16
    q = np.floor(np.arange(N, dtype=np.float32) / 2.0) * 2.0
    freqs = (1.0 / (theta ** (q / np.float32(N))) / np.float32(2.0 * math.pi)).astype(np.float32)
    phases = np.arange(T, dtype=np.float32)[:, None] * freqs[None, 0::2]
    ph = np.float32(2.0 * math.pi) * (phases % np.float32(1.0))
    ctab = bf(np.cos(ph).T)
    stab = bf(np.sin(ph).T)

    x0 = np.ascontiguousarray(embed_w[idx[0]]).astype(np.float32)
    lm_bf = bf(lm_head)

    r = np.arange(128)[:, None]
    c64 = np.arange(64)[None, :]
    in_maps = []
    for c in range(N_CORES):
        h, p = c // 2, c % 2
        m01 = np.zeros((128, 2), np.float32)
        m01[:, 0] = 1.0 - p
        m01[:, 1] = p
        in_maps.append({
            "enc": bf(encoder[h][:, perm]),
            "encv": bf(encoder_v[h][:, perm]),
            "dec": bf(decoder.reshape(NH, N, D)[h][perm, :]),
            "lm": lm_bf,
            "ctab": ctab,
            "stab": stab,
            "mask": bf((r < c64 + 64 * p).astype(np.float32)),
            "m01": m01,
            "poff": np.array([[64 * p]], dtype=np.uint32),
            "x0": x0,
        })
    return in_maps


_NC_CACHE = {}


def kernel(idx, n_layer, embed_w, encoder, encoder_v, decoder, lm_head,
           _trace=False, _trace_kwargs=None):
    n_layer = int(np.asarray(n_layer))
    idx = np.asarray(idx)
    B = idx.shape[0]
    if n_layer not in _NC_CACHE:
        _NC_CACHE[n_layer] = build(n_layer)
    nc = _NC_CACHE[n_layer]
    in_maps = _host_prep(idx, np.asarray(embed_w, np.float32),
                         np.asarray(encoder, np.float32),
                         np.asarray(encoder_v, np.float32),
                         np.asarray(decoder, np.float32),
                         np.asarray(lm_head, np.float32))
    kw = {}
    if _trace:
        kw = dict(trace=True, **(_trace_kwargs or {}))
    res = run_bass_kernel_spmd(nc, in_maps, core_ids=list(range(N_CORES)), **kw)
    logits = res.results[0]["out"].astype(np.float32).reshape(B, T, VOCAB)
    kernel._last_results = res
    return logits



# revision 39
# speedup vs baseline: 1.1583x; 1.0049x over previous
"""BDH (dense_transformer) Trainium2 kernel, 8-core tensor-parallel.

Sharding: core c -> head h=c//2, parity p=c%2. Within a head, the two cores
split the T=1024 query dim into 16 blocks of 64 (block B=2j+p, j=0..7) for
causal load balance. The N=4096 latent dim is host-permuted to [evens, odds]
so rope is pure elementwise between the E half and the O half.

Each layer is software-pipelined over causal T-halves: queries t<512 attend
only keys t<512, so half 0's full pipeline (encoder+rope -> fused scores ->
yKV -> y_sparse/decoder) finishes and launches its fp16 AllReduce roughly
half a layer before the layer ends; both collectives are consumed a full
half-layer after launch, hiding the ~15-30us mesh-collective latency behind
compute. Half 1's encoder+rope (which only needs the row-4..7 residual
updates) is interleaved into half 0's PE-light decoder window; its score
matmuls run as a standalone pass. Residual updates for rows 0..3 / 4..7 are
slotted where their AllReduce is provably complete, because a dependent DMA
wait head-of-line blocks the whole in-order engine queue hosting it. Score
accumulators pack two causal-suffix regions per PSUM bank ('start' may only
be set on a bank's first matmul - it clears has_written bank-wide). The
residual stream lives in bf16; x_sparse spills to DRAM in batched 4-tile
descriptors and is re-read the same way.

kernel(**inputs) takes full unsharded inputs, returns full (B,T,vocab) logits.
"""
import math
import sys

sys.path.insert(0, "/opt/trn_rl_repo")

import numpy as np
import ml_dtypes

import concourse.bass as bass
import concourse.mybir as mybir
import concourse.tile as tile
from concourse import bacc
from concourse.masks import make_identity
from concourse.tile import add_dep_helper
from concourse.bass_utils import run_bass_kernel_spmd

FP32 = mybir.dt.float32
FP16 = mybir.dt.float16
BF16 = mybir.dt.bfloat16
AF = mybir.ActivationFunctionType
ALU = mybir.AluOpType

N_CORES = 8
T = 1024
D = 256
NH = 4
N = 4096
HALF = N // 2
VOCAB = 256
EPS = 1e-5
NT = N // 128
NPAIR = HALF // 128    # 16 E/O tile pairs
TT8 = T // 128
QCOLS = 512


def build(n_layer: int):
    nc = bacc.Bacc("TRN2", target_bir_lowering=False, debug=False,
                   num_devices=N_CORES)

    enc_in = nc.dram_tensor("enc", [D, N], BF16, kind="ExternalInput").ap()
    encv_in = nc.dram_tensor("encv", [D, N], BF16, kind="ExternalInput").ap()
    dec_in = nc.dram_tensor("dec", [N, D], BF16, kind="ExternalInput").ap()
    lm_in = nc.dram_tensor("lm", [D, VOCAB], BF16, kind="ExternalInput").ap()
    ctab_in = nc.dram_tensor("ctab", [HALF, T], BF16, kind="ExternalInput").ap()
    stab_in = nc.dram_tensor("stab", [HALF, T], BF16, kind="ExternalInput").ap()
    mask_in = nc.dram_tensor("mask", [128, 64], BF16, kind="ExternalInput").ap()
    m01_in = nc.dram_tensor("m01", [128, 2], FP32, kind="ExternalInput").ap()
    poff_in = nc.dram_tensor("poff", [1, 1], mybir.dt.uint32, kind="ExternalInput").ap()
    x0_in = nc.dram_tensor("x0", [T, D], FP32, kind="ExternalInput").ap()
    out = nc.dram_tensor("out", [T, VOCAB], FP32, kind="ExternalOutput").ap()

    RG = [list(range(N_CORES))]

    with tile.TileContext(nc) as tc:
        regs = nc.alloc_registers("qoff")
        nc.regs_load(regs, poff_in[0:1, 0:1])
        qoff = nc.snap(regs, donate=True, min_val=0, max_val=64)

        import contextlib
        ctx = contextlib.ExitStack()
        with ctx:
            singles = ctx.enter_context(tc.tile_pool(name="singles", bufs=1))
            big = ctx.enter_context(tc.tile_pool(name="big", bufs=1))
            wide = ctx.enter_context(tc.tile_pool(name="wide", bufs=2))
            tmpp = ctx.enter_context(tc.tile_pool(name="tmpp", bufs=1))
            small = ctx.enter_context(tc.tile_pool(name="small", bufs=2))
            stat = ctx.enter_context(tc.tile_pool(name="stat", bufs=3))
            ps = ctx.enter_context(tc.tile_pool(name="ps", bufs=1, space="PSUM"))
            dramp = ctx.enter_context(tc.tile_pool(name="dramp", bufs=2, space="DRAM"))
            spillp = ctx.enter_context(tc.tile_pool(name="spillp", bufs=34, space="DRAM"))

            # ---- persistent weights ----
            enc_sb = singles.tile([128, 2, N], BF16)
            encv_sb = singles.tile([128, 2, N], BF16)
            dec_sb = singles.tile([128, NT, D], BF16)
            lm_sb = singles.tile([128, 2, VOCAB], BF16)
            mask_sb = singles.tile([128, 64], BF16)
            m01_sb = singles.tile([128, 2], FP32)
            eps_sb = singles.tile([128, 1], FP32)
            ident = singles.tile([128, 128], BF16)
            nc.sync.dma_start(out=enc_sb, in_=enc_in.rearrange("(kt p) n -> p kt n", p=128))
            nc.sync.dma_start(out=encv_sb, in_=encv_in.rearrange("(kt p) n -> p kt n", p=128))
            nc.sync.dma_start(out=dec_sb, in_=dec_in.rearrange("(nt p) d -> p nt d", p=128))
            nc.sync.dma_start(out=lm_sb, in_=lm_in.rearrange("(kt p) v -> p kt v", p=128))
            nc.sync.dma_start(out=mask_sb, in_=mask_in)
            nc.sync.dma_start(out=m01_sb, in_=m01_in)
            nc.vector.memset(eps_sb, EPS)
            make_identity(nc, ident)

            # ---- persistent activations ----
            xbf_sb = big.tile([128, TT8, D], BF16)
            xT_lo = big.tile([128, 2, 512], BF16)   # t cols 0..511 (d-part)
            xT_hi = big.tile([128, 2, 512], BF16)   # t cols 512..1023
            qr_sb = big.tile([128, NT, T], BF16)
            a_sb = big.tile([128, TT8, QCOLS], BF16)
            ykv_sb = big.tile([128, 4, D], BF16)
            ykvT_sb = big.tile([128, 2, QCOLS], BF16)

            def layernorm(dst, src, tag=""):
                stats = stat.tile([128, 6], FP32, tag="lnstats" + tag)
                mv = stat.tile([128, 2], FP32, tag="lnmv" + tag)
                nc.vector.bn_stats(out=stats, in_=src)
                nc.vector.bn_aggr(out=mv, in_=stats)
                std_t = stat.tile([128, 1], FP32, tag="lnstd" + tag)
                nc.scalar.activation(out=std_t, in_=mv[:, 1:2], func=AF.Sqrt,
                                     bias=eps_sb, scale=1.0)
                rstd = stat.tile([128, 1], FP32, tag="lnrstd" + tag)
                nc.vector.reciprocal(out=rstd, in_=std_t)
                negmr = stat.tile([128, 1], FP32, tag="lnnegmr" + tag)
                nc.vector.tensor_scalar(out=negmr, in0=mv[:, 0:1], scalar1=rstd,
                                        scalar2=-1.0, op0=ALU.mult, op1=ALU.mult)
                nc.scalar.activation(out=dst, in_=src, func=AF.Identity,
                                     bias=negmr, scale=rstd)

            def x_finalize(ti):
                xT = xT_lo if ti < 4 else xT_hi
                base = 128 * (ti % 4)
                for dh in range(2):
                    pst = ps.tile([128, 128], BF16, tag=f"b{6 + dh}", name=f"tp{ti}_{dh}")
                    nc.tensor.transpose(pst[:], xbf_sb[:, ti, 128 * dh:128 * dh + 128], ident[:])
                    nc.scalar.copy(out=xT[:, dh, base:base + 128], in_=pst[:])

            spill_map = {}
            GS = 4            # pairs per rope group
            NG = NPAIR // GS  # 4 groups per chunk

            def phaseA(ell, ch, unit0, cols=(0, 512), slots=None, fused=None,
                       rot=tuple(range(8)), groups=None):
                """x_sparse+rope for t-chunk ch over query-column range cols, in
                groups of GS=4 n-tile pairs so the rope runs as wide strided DVE
                ops and the spill/table DMAs batch 4 pairs per descriptor. unit0
                offsets the psum tag rotation. slots: optional {g: callback}
                invoked before group g (x_update interleaving). fused: optional
                (half, s_ps) - score matmuls for that causal T-half accumulate
                into the s_ps psum slices as each group completes."""
                xTc = xT_lo if ch == 0 else xT_hi
                c0, cw = cols
                for g in (range(NG) if groups is None else groups):
                    if slots is not None and g in slots:
                        slots[g]()
                    xsE2 = wide.tile([128, GS, cw], BF16, tag="xsE")
                    xsO2 = wide.tile([128, GS, cw], BF16, tag="xsO")
                    if True:
                        for gi in range(GS):
                            i = GS * g + gi
                            u = (unit0 + 2 * i) % len(rot)
                            tagE = f"b{rot[u]}"
                            tagO = f"b{rot[(u + 1) % len(rot)]}"
                            psE = ps.tile([128, cw], FP32, tag=tagE, name=f"psE{ell}_{ch}_{c0}_{i}")
                            psO = ps.tile([128, cw], FP32, tag=tagO, name=f"psO{ell}_{ch}_{c0}_{i}")
                            for kt in range(2):
                                nc.tensor.matmul(psE[:], enc_sb[:, kt, 128 * i:128 * i + 128],
                                                 xTc[:, kt, c0:c0 + cw], start=(kt == 0), stop=(kt == 1))
                            for kt in range(2):
                                nc.tensor.matmul(psO[:], enc_sb[:, kt, HALF + 128 * i:HALF + 128 * i + 128],
                                                 xTc[:, kt, c0:c0 + cw], start=(kt == 0), stop=(kt == 1))
                            nc.scalar.activation(out=xsE2[:, gi, :], in_=psE[:], func=AF.Relu)
                            nc.scalar.activation(out=xsO2[:, gi, :], in_=psO[:], func=AF.Relu)
                    if ch == 0 and c0 == 0:
                        spill_map[(ell, g)] = spillp.tile([128, GS, QCOLS], BF16, tag="xsq",
                                                          name=f"sp{ell}_E{g}")
                        spill_map[(ell, NG + g)] = spillp.tile([128, GS, QCOLS], BF16,
                                                               tag="xsq", name=f"sp{ell}_O{g}")
                    q0 = 256 * ch + c0 // 2
                    for half, xs2 in ((0, xsE2), (1, xsO2)):
                        sp = spill_map[(ell, NG * half + g)]
                        nc.gpsimd.dma_start(
                            out=sp[:, :, q0:q0 + cw // 2].rearrange(
                                "p g (b w) -> p g b w", w=64),
                            in_=xs2[:].rearrange("p g (b w) -> p g b w", w=128)[:, :, :, bass.ds(qoff, 64)])
                    ct2 = small.tile([128, GS, cw], BF16, tag="ctab")
                    st2 = small.tile([128, GS, cw], BF16, tag="stab")
                    tc0 = 512 * ch + c0
                    nc.sync.dma_start(
                        out=ct2, in_=ctab_in[512 * g:512 * g + 512, tc0:tc0 + cw]
                        .rearrange("(g p) w -> p g w", p=128))
                    nc.sync.dma_start(
                        out=st2, in_=stab_in[512 * g:512 * g + 512, tc0:tc0 + cw]
                        .rearrange("(g p) w -> p g w", p=128))
                    qrE = qr_sb[:, GS * g:GS * g + GS, tc0:tc0 + cw]
                    qrO = qr_sb[:, 16 + GS * g:16 + GS * g + GS, tc0:tc0 + cw]
                    tB = tmpp.tile([128, GS, cw], BF16, tag="tt")
                    nc.vector.tensor_tensor(out=qrO, in0=xsO2[:], in1=ct2[:], op=ALU.mult)
                    nc.vector.tensor_tensor(out=tB, in0=xsE2[:], in1=st2[:], op=ALU.mult)
                    nc.vector.tensor_tensor(out=qrO, in0=qrO, in1=tB, op=ALU.add)
                    tD = tmpp.tile([128, GS, cw], BF16, tag="tt")
                    nc.vector.tensor_tensor(out=qrE, in0=xsE2[:], in1=ct2[:], op=ALU.mult)
                    nc.vector.tensor_tensor(out=tD, in0=xsO2[:], in1=st2[:], op=ALU.mult)
                    nc.vector.tensor_tensor(out=qrE, in0=qrE, in1=tD, op=ALU.subtract)
                    if fused is not None:
                        h, s_ps = fused
                        tiles = ([GS * g + k for k in range(GS)]
                                 + [16 + GS * g + k for k in range(GS)])
                        bmax = 4 * (h + 1)
                        for tile_n in tiles:
                            for t in range(4 * h + 4):
                                b0 = max(t, 4 * h)
                                rhs = qr_sb[:, tile_n, :].rearrange(
                                    "p (b w) -> p b w", w=128)[:, b0:bmax, bass.ds(qoff, 64)]
                                # s_ps packs two t-regions per psum bank; start
                                # (which clears has_written for the WHOLE bank)
                                # may only be set on the bank's first matmul.
                                # The odd-t region's first write then overwrites
                                # (its has_written bits are clear) and later
                                # matmuls accumulate.
                                nc.tensor.matmul(
                                    s_ps[t].rearrange("p (b w) -> p b w", w=64),
                                    qr_sb[:, tile_n, 128 * t:128 * t + 128],
                                    rhs, start=(g == 0 and tile_n == 0 and t % 2 == 0),
                                    stop=(g == NG - 1 and tile_n == 16 + NPAIR - 1))
                if slots is not None and NG in slots:
                    slots[NG]()

            # warmup collective: absorbs the one-time global sync barrier
            wu_in = dramp.tile([512, D], FP16, tag="wuin")
            wu_out = dramp.tile([512, D], FP16, tag="wuout")
            wu_sb = singles.tile([128, 4 * D], FP16)
            nc.vector.memset(wu_sb, 0.0)
            nc.sync.dma_start(out=wu_in[:].rearrange("(a p) d -> p a d", p=128),
                              in_=wu_sb[:].rearrange("p (a d) -> p a d", d=D))
            nc.gpsimd.collective_compute("AllReduce", ALU.add, replica_groups=RG,
                                         ins=[wu_in.opt()], outs=[wu_out.opt()])

            # ---- prologue ----
            # a_sb's strict-upper zero region [0, 64t) per t-block is never
            # rewritten by the per-layer score assembly, so zero it once.
            nc.vector.memset(a_sb[:], 0.0)
            for ti in range(TT8):
                raw = small.tile([128, D], FP32, tag="x0raw")
                nc.sync.dma_start(out=raw, in_=x0_in[128 * ti:128 * ti + 128, :])
                layernorm(xbf_sb[:, ti, :], raw)
                x_finalize(ti)

            def make_sps0(ell):
                pa = ps.tile([128, 448], FP32, tag="b4", name=f"sps0a{ell}")
                pb = ps.tile([128, 192], FP32, tag="b5", name=f"sps0b{ell}")
                return {0: pa[:, 0:256], 1: pa[:, 256:448],
                        2: pb[:, 0:128], 3: pb[:, 128:192]}

            def make_sps1(ell):
                p01 = ps.tile([128, 512], FP32, tag="b0", name=f"sps1a{ell}")
                p23 = ps.tile([128, 512], FP32, tag="b1", name=f"sps1b{ell}")
                p45 = ps.tile([128, 448], FP32, tag="b2", name=f"sps1c{ell}")
                p67 = ps.tile([128, 192], FP32, tag="b3", name=f"sps1d{ell}")
                return {0: p01[:, 0:256], 1: p01[:, 256:512],
                        2: p23[:, 0:256], 3: p23[:, 256:512],
                        4: p45[:, 0:256], 5: p45[:, 256:448],
                        6: p67[:, 0:128], 7: p67[:, 128:192]}

            def assembly(h, s_ps):
                """a_sb local-query columns [256h, 256h+256) from packed score
                psums: masked diagonal 64-block + causal-suffix copy."""
                if h == 0:
                    for t in range(4):
                        w = (4 - t) * 64
                        nc.vector.tensor_tensor(out=a_sb[:, t, 64 * t:64 * t + 64],
                                                in0=s_ps[t][:, 0:64], in1=mask_sb,
                                                op=ALU.mult)
                        if w > 64:
                            nc.scalar.copy(out=a_sb[:, t, 64 * t + 64:256],
                                           in_=s_ps[t][:, 64:w])
                else:
                    for t in range(4):
                        nc.scalar.copy(out=a_sb[:, t, 256:512], in_=s_ps[t][:, 0:256])
                    for t in range(4, 8):
                        w = (8 - t) * 64
                        nc.vector.tensor_tensor(out=a_sb[:, t, 64 * t:64 * t + 64],
                                                in0=s_ps[t][:, 0:64], in1=mask_sb,
                                                op=ALU.mult)
                        if w > 64:
                            nc.scalar.copy(out=a_sb[:, t, 64 * t + 64:512],
                                           in_=s_ps[t][:, 64:w])

            def phaseC_k(k, ell):
                psy = ps.tile([128, D], FP32, tag=f"b{4 * (k // 2) + (k % 2)}",
                              name=f"ykvps{ell}_{k}")
                tmax = 2 * k + 1
                for t in range(tmax + 1):
                    nc.tensor.matmul(psy[:], a_sb[:, t, 128 * k:128 * k + 128],
                                     xbf_sb[:, t, :], start=(t == 0), stop=(t == tmax))
                layernorm(ykv_sb[:, k, :], psy[:])
                for dh in range(2):
                    pst = ps.tile([128, 128], BF16, tag=f"b{6 + dh}",
                                  name=f"ykvT{ell}_{k}_{dh}")
                    nc.tensor.transpose(pst[:], ykv_sb[:, k, 128 * dh:128 * dh + 128],
                                        ident[:])
                    nc.scalar.copy(out=ykvT_sb[:, dh, 128 * k:128 * k + 128], in_=pst[:])

            def phaseC(h, ell):
                for k in (2 * h, 2 * h + 1):
                    phaseC_k(k, ell)

            def phaseD(h, ell, psmlp, upds=(), upd_at=(), dr=(0, 1)):
                """upds[i] is emitted before loop iteration upd_at[i]; an
                x_update whose AllReduce may still be in flight head-of-line
                blocks the in-order queues, so slots sit late enough that the
                collective has drained by the time the queues reach them."""
                q0 = 256 * h
                xsq4 = None
                for idx, jp in enumerate(range(0, NT, 2)):
                    if idx in upd_at:
                        upds[upd_at.index(idx)]()
                    ys2 = small.tile([128, 2, D], BF16, tag="ys")
                    # two y_sparse chains share one psum bank (has_written:
                    # start only on the bank's first matmul)
                    psy = ps.tile([128, 2, D], FP32, tag=f"b{dr[(jp // 2) % len(dr)]}",
                                  name=f"ysps{ell}_{h}_{jp}")
                    for jj in range(2):
                        j = jp + jj
                        for kt in range(2):
                            nc.tensor.matmul(psy[:, jj, :], encv_sb[:, kt, 128 * j:128 * j + 128],
                                             ykvT_sb[:, kt, q0:q0 + 256],
                                             start=(kt == 0 and jj == 0), stop=(kt == 1))
                    nc.scalar.activation(out=ys2[:], in_=psy[:], func=AF.Relu)
                    if jp < 16:
                        gsp, sl = jp // GS, jp % GS
                    else:
                        gsp, sl = NG + (jp - 16) // GS, (jp - 16) % GS
                    if sl % 4 == 0:
                        xsq4 = small.tile([128, GS, D], BF16, tag="xsqb")
                        nc.sync.dma_start(out=xsq4,
                                          in_=spill_map[(ell, gsp)][:, :, q0:q0 + 256])
                    xy2 = small.tile([128, 2, D], BF16, tag="xy")
                    nc.vector.tensor_tensor(out=xy2, in0=ys2,
                                            in1=xsq4[:, sl % 4:sl % 4 + 2, :], op=ALU.mult)
                    for jj in range(2):
                        for kk in range(2):
                            nc.tensor.matmul(psmlp[:, kk, :],
                                             xy2[:, jj, 128 * kk:128 * kk + 128],
                                             dec_sb[:, jp + jj, :],
                                             start=(jp + jj == 0 and kk == 0),
                                             stop=(jp + jj == NT - 1))


            def phaseE(h, ell, psmlp):
                """fp16 AllReduce of this half's yMLP partial (global rows
                [512h, 512h+512)); foreign-parity 64-blocks written as zeros."""
                arin = dramp.tile([512, D], FP16, tag=f"ar{h}in", name=f"arin{ell}_{h}")
                arout = dramp.tile([512, D], FP16, tag=f"ar{h}out", name=f"arout{ell}_{h}")
                wds = []
                for P in (0, 1):
                    for kk in range(2):
                        ym = stat.tile([128, D], FP16, tag="ymsk")
                        nc.vector.tensor_scalar_mul(out=ym, in0=psmlp[:, kk, :],
                                                    scalar1=m01_sb[:, P:P + 1])
                        for aa in range(2):
                            out_ap = bass.AP(
                                tensor=arin.tensor,
                                offset=arin.offset + (256 * kk + 64 * P + 128 * aa) * D,
                                ap=[[D, 64], [1, D]],
                            )
                            eng = (nc.sync, nc.gpsimd)[aa]
                            wds.append(eng.dma_start(
                                out=out_ap, in_=ym[64 * aa:64 * aa + 64, :]))
                cc = nc.gpsimd.collective_compute(
                    "AllReduce", ALU.add, replica_groups=RG,
                    ins=[arin.opt()], outs=[arout.opt()])
                for w in wds:
                    add_dep_helper(cc.ins, w.ins, sync=True, reason="w->ar")
                return cc, arout

            def x_update(ti, ccpair):
                cc, arout = ccpair
                rd = stat.tile([128, D], FP16, tag="ymrd")
                # issue on the Vector queue: the AllReduce-completion wait then
                # head-of-line blocks only work that depends on this update,
                # never the table/spill DMA stream on Sync.
                rdma = nc.sync.dma_start(
                    out=rd, in_=arout[128 * (ti % 4):128 * (ti % 4) + 128, :])
                add_dep_helper(rdma.ins, cc.ins, sync=True, reason="ar->r")
                lnym = stat.tile([128, D], FP32, tag="lnym")
                layernorm(lnym, rd)
                xn = stat.tile([128, D], FP32, tag="xn")
                nc.vector.tensor_add(out=xn, in0=xbf_sb[:, ti, :], in1=lnym)
                layernorm(xbf_sb[:, ti, :], xn)
                x_finalize(ti)

            def make_a0g(lell):
                return tuple(
                    (lambda g=g: phaseA(lell, 0, 0, cols=(0, 512),
                                        rot=(2, 3, 5), groups=(g,)))
                    for g in range(NG))

            # ---- layers, software-pipelined over causal T-halves ----
            # Half 0 (queries t<512) only attends keys t<512, so its entire
            # pipeline A..E runs and its AllReduce launches ~half a layer before
            # the layer ends; both collectives are consumed a full half-layer
            # after launch, hiding the ~15us collective latency completely.
            ar1_prev = None
            for ell in range(n_layer):
                # -- half 0 --
                # A0's encoder+rope groups: 0..2 were interleaved into the
                # previous layer's D1 window; the last one runs here.
                if ell == 0:
                    for f in make_a0g(0):
                        f()
                else:
                    a0g_carry[3]()
                s_ps0 = make_sps0(ell)
                for tile_n in range(NT):
                    if ell > 0 and tile_n in (8, 16, 24):
                        x_update(4 + (tile_n - 8) // 8, ar1_prev)
                    for t in range(4):
                        rhs = qr_sb[:, tile_n, :].rearrange(
                            "p (b w) -> p b w", w=128)[:, t:4, bass.ds(qoff, 64)]
                        nc.tensor.matmul(
                            s_ps0[t].rearrange("p (b w) -> p b w", w=64),
                            qr_sb[:, tile_n, 128 * t:128 * t + 128],
                            rhs, start=(tile_n == 0 and t % 2 == 0),
                            stop=(tile_n == NT - 1))
                if ell > 0:
                    x_update(7, ar1_prev)
                assembly(0, s_ps0)
                phaseC(0, ell)
                # A1's encoder+rope only needs the row-4..7 updates, so its
                # groups interleave into the PE-light D0 window; its score
                # matmuls run as a standalone pass afterwards.
                a1g = tuple(
                    (lambda g=g: phaseA(ell, 1, 0, cols=(0, 512),
                                        rot=(2, 3, 5, 6, 7), groups=(g,)))
                    for g in range(NG))
                psmlp0 = ps.tile([128, 2, D], FP32, tag="b4", name=f"psmlp{ell}_0")
                phaseD(0, ell, psmlp0, a1g[:3], upd_at=(4, 8, 12))
                cc0 = phaseE(0, ell, psmlp0)
                # -- half 1 --
                a1g[3]()
                s_ps1 = make_sps1(ell)
                for tile_n in range(NT):
                    for t in range(8):
                        b0 = max(t, 4)
                        rhs = qr_sb[:, tile_n, :].rearrange(
                            "p (b w) -> p b w", w=128)[:, b0:TT8, bass.ds(qoff, 64)]
                        nc.tensor.matmul(
                            s_ps1[t].rearrange("p (b w) -> p b w", w=64),
                            qr_sb[:, tile_n, 128 * t:128 * t + 128],
                            rhs, start=(tile_n == 0 and t % 2 == 0),
                            stop=(tile_n == NT - 1))
                assembly(1, s_ps1)
                phaseC(1, ell)
                psmlp1 = ps.tile([128, 2, D], FP32, tag="b4", name=f"psmlp{ell}_1")
                d1_slots = [
                    (lambda ti=ti, cp=cc0: x_update(ti, cp)) for ti in (0, 1, 2, 3)]
                d1_at = [0, 2, 4, 6]
                if ell + 1 < n_layer:
                    a0g_carry = make_a0g(ell + 1)
                    d1_slots += list(a0g_carry[:3])
                    d1_at += [8, 11, 14]
                phaseD(1, ell, psmlp1, tuple(d1_slots), upd_at=tuple(d1_at))
                ar1_prev = phaseE(1, ell, psmlp1)

            # ---- logits ----
            def logits(ti):
                psl = ps.tile([128, VOCAB], FP32, tag=f"b{ti % 2}", name=f"lgps{ti}")
                xT = xT_lo if ti < 4 else xT_hi
                base = 128 * (ti % 4)
                for kt in range(2):
                    nc.tensor.matmul(psl[:], xT[:, kt, base:base + 128],
                                     lm_sb[:, kt, :], start=(kt == 0), stop=(kt == 1))
                lg = small.tile([128, VOCAB], FP32, tag="lg")
                nc.scalar.copy(out=lg, in_=psl[:])
                nc.sync.dma_start(out=out[128 * ti:128 * ti + 128, :], in_=lg)

            for ti in range(4):
                logits(ti)
            for ti in range(4, 8):
                x_update(ti, ar1_prev)
            for ti in range(4, 8):
                logits(ti)

    nc.compile()
    return nc


def _host_prep(idx, embed_w, encoder, encoder_v, decoder, lm_head):
    idx = np.asarray(idx)
    B, Tt = idx.shape
    assert B == 1 and Tt == T
    perm = np.concatenate([np.arange(0, N, 2), np.arange(1, N, 2)])

    def bf(x):
        return np.ascontiguousarray(x).astype(ml_dtypes.bfloat16)

    theta = 2.0 ** 16
    q = np.floor(np.arange(N, dtype=np.float32) / 2.0) * 2.0
    freqs = (1.0 / (theta ** (q / np.float32(N))) / np.float32(2.0 * math.pi)).astype(np.float32)
    phases = np.arange(T, dtype=np.float32)[:, None] * freqs[None, 0::2]
    ph = np.float32(2.0 * math.pi) * (phases % np.float32(1.0))
    ctab = bf(np.cos(ph).T)
    stab = bf(np.sin(ph).T)

    x0 = np.ascontiguousarray(embed_w[idx[0]]).astype(np.float32)
    lm_bf = bf(lm_head)

    r = np.arange(128)[:, None]
    c64 = np.arange(64)[None, :]
    in_maps = []
    for c in range(N_CORES):
        h, p = c // 2, c % 2
        m01 = np.zeros((128, 2), np.float32)
        m01[:, 0] = 1.0 - p
        m01[:, 1] = p
        in_maps.append({
            "enc": bf(encoder[h][:, perm]),
            "encv": bf(encoder_v[h][:, perm]),
            "dec": bf(decoder.reshape(NH, N, D)[h][perm, :]),
            "lm": lm_bf,
            "ctab": ctab,
            "stab": stab,
            "mask": bf((r < c64 + 64 * p).astype(np.float32)),
            "m01": m01,
            "poff": np.array([[64 * p]], dtype=np.uint32),
            "x0": x0,
        })
    return in_maps


_NC_CACHE = {}


def kernel(idx, n_layer, embed_w, encoder, encoder_v, decoder, lm_head,
           _trace=False, _trace_kwargs=None):
    n_layer = int(np.asarray(n_layer))
    idx = np.asarray(idx)
    B = idx.shape[0]
    if n_layer not in _NC_CACHE:
        _NC_CACHE[n_layer] = build(n_layer)
    nc = _NC_CACHE[n_layer]
    in_maps = _host_prep(idx, np.asarray(embed_w, np.float32),
                         np.asarray(encoder, np.float32),
                         np.asarray(encoder_v, np.float32),
                         np.asarray(decoder, np.float32),
                         np.asarray(lm_head, np.float32))
    kw = {}
    if _trace:
        kw = dict(trace=True, **(_trace_kwargs or {}))
    res = run_bass_kernel_spmd(nc, in_maps, core_ids=list(range(N_CORES)), **kw)
    logits = res.results[0]["out"].astype(np.float32).reshape(B, T, VOCAB)
    kernel._last_results = res
    return logits

